# revision 1
# baseline (speedup 1.0000x reference)
"""Distributed Trainium2 Bass kernel for nn_NodeFeat (2-hop Chebyshev-style GNN
feature expansion + edge gather), 8 NeuronCores.

Node sharding per the problem's sharding hint:
  - 50000 nodes padded to 50176 = 8 x 6272; core c owns rows [6272c, 6272c+6272).
  - adjacency rows are pre-sorted; each core handles the edges whose ROW is in
    its shard, packed per 128-row tile into NCHUNK=18 chunks of 128 slots
    (dummy slots use an out-of-bounds index -> DMA descriptor skipped).
  - hop1: indirect-DMA gather of x[col] rows, scaled per-edge by
    {1, rsqrt(deg_col), sqrt(deg_col)} into a [128,192] fp16 moving operand;
    segment-sum on TensorE via a one-hot selector (is_equal of rowloc vs iota)
    accumulated in PSUM; ScalarE evacuates with the 1/deg row scale.
  - one on-chip AllGather of the per-core y1 shard between hops.
  - hop2: same machinery gathering y1 rows, then minus xs0.
  - final: edge endpoints partitioned by owner core (host all-to-all
    bookkeeping); each core gathers its [xs0|y1|xs2] rows, transposes [9,64]
    -> [64,9] on-chip, writes packed rows; host scatters into [2,32768,64,9].

All floating-point math runs on device; the host only shards, pads, reorders
and reassembles (index bookkeeping).
"""
import numpy as np

import concourse.bass as bass
import concourse.mybir as mybir
import concourse.tile as tile
from concourse.bass_utils import run_bass_kernel_spmd

# ---------------- hardcoded problem geometry ----------------
N = 50000
D = 64
EQ = 32768
P = 128
NC = 8                   # cores
NT = 49                  # row tiles per core
NSH = NT * P             # 6272 rows per core
NPAD = NSH * NC          # 50176
NCHUNK = 18              # 128-edge chunks per row tile
FCH = 66                 # final-gather chunks per core (66*128 = 8448 slots)
PC = 6                   # final-gather chunks per piece (11 pieces)
BIG = 10 ** 7            # out-of-bounds index -> DMA descriptor skipped
F32 = mybir.dt.float32
F16 = mybir.dt.float16
I32 = mybir.dt.int32
EDGE_COLS = NT * NCHUNK  # 882

_prog_cache = {}


class _TC(tile.TileContext):
    """TileContext whose final drain splits sem waits one-per-instruction
    (this walrus rejects >1 sync wait on an instruction)."""

    def _drain_and_barrier(self, tick_clock, wait_clock):
        nc = self.nc
        probe = nc.sync.nop()
        wait_clock.add_sem_waits(
            probe.ins, tile.ScopedClock({None: tick_clock.global_clock}))
        si = probe.ins.sync_info
        waits = list(si.on_wait) if si and si.on_wait else []
        if si is not None:
            si.on_wait = waits[:1]
        for w in waits[1:]:
            n2 = nc.sync.nop()
            if n2.ins.sync_info is None:
                n2.ins.sync_info = mybir.SyncInfo(on_wait=[w], on_update=[])
            else:
                n2.ins.sync_info.on_wait = [w]
        nc.sync.drain()
        nc.all_engine_barrier()
        popped = nc._tile_sem_poison_stack.pop()
        assert popped is self._sem_poison
        nc.clear_and_free_semaphores(list(self.sems.allocated().values()))
        nc.all_engine_barrier()


def _split_multi_waits(nc):
    for fn in nc.m.functions:
        for blk in fn.blocks:
            new_list = []
            for inst in blk.instructions:
                si = inst.sync_info
                waits = list(si.on_wait) if si and si.on_wait else []
                if len(waits) > 1:
                    for j, w in enumerate(waits[:-1]):
                        nop = mybir.InstNoOp(
                            name=f"{inst.name}-ws{j}",
                            engine=inst.engine,
                            ins=[], outs=[],
                            sync_info=mybir.SyncInfo(on_wait=[w], on_update=[]),
                        )
                        nc.register_instruction(nop, overwrite=True)
                        new_list.append(nop)
                    si.on_wait = waits[-1:]
                new_list.append(inst)
            blk.instructions[:] = new_list


def _dims(ap, dims):
    """Same tensor+offset as `ap`, explicit [stride(elem), nelem] dims."""
    return bass.AP(ap.tensor, ap.offset, dims)


def _build_program(ablate=()):
    """ablate: subset of {"hop1","gather1","ag","hop2","gather2","final","gatherf"}
    to SKIP (for performance ablation only — results become wrong)."""
    ab = set(ablate)
    nc = bass.Bass("TRN2", target_bir_lowering=False, debug=False, num_devices=NC)

    x_full = nc.dram_tensor("x_full", [NPAD, D], F32, kind="ExternalInput")
    x_sh = nc.dram_tensor("x_sh", [NSH, D], F32, kind="ExternalInput")
    degsh_in = nc.dram_tensor("degsh", [P, NT], F32, kind="ExternalInput")
    idx1_in = nc.dram_tensor("idx1", [P, EDGE_COLS], I32, kind="ExternalInput")
    rowloc_in = nc.dram_tensor("rowloc", [P, EDGE_COLS], F16, kind="ExternalInput")
    degcol_in = nc.dram_tensor("degcol", [P, EDGE_COLS], F32, kind="ExternalInput")
    fidx_loc_in = nc.dram_tensor("fidx_loc", [P, FCH], I32, kind="ExternalInput")
    fidx_mid_in = nc.dram_tensor("fidx_mid", [P, FCH], I32, kind="ExternalInput")
    iota_in = nc.dram_tensor("iota", [P, P], F16, kind="ExternalInput")

    out_f = nc.dram_tensor("out_f", [FCH * P, 576], F32, kind="ExternalOutput")

    y1_bounce = nc.dram_tensor("y1_bounce", [NSH, 192], F32)
    y1full = nc.dram_tensor("y1full", [NPAD, 192], F32, addr_space="Shared")
    xs0_l = nc.dram_tensor("xs0_l", [NSH, 192], F32)
    xs2_l = nc.dram_tensor("xs2_l", [NSH, 192], F32)

    eq = mybir.AluOpType.is_equal
    mult = mybir.AluOpType.mult
    sub = mybir.AluOpType.subtract
    COPY = mybir.ActivationFunctionType.Copy
    SQRT = mybir.ActivationFunctionType.Sqrt

    with _TC(nc) as tc, nc.allow_low_precision(reason="fp16 matmul operands; PSUM accumulates in f32"), \
            nc.gpsimd.register("bnd_pad") as bnd_pad, \
            nc.gpsimd.register("bnd_sh") as bnd_sh:
        nc.gpsimd.reg_mov(bnd_pad, NPAD - 1)
        nc.gpsimd.reg_mov(bnd_sh, NSH - 1)
        with (
            tc.tile_pool(name="const", bufs=1) as cp,
            tc.tile_pool(name="v1", bufs=3) as v1p,
            tc.tile_pool(name="s", bufs=3) as sp_,
            tc.tile_pool(name="v3", bufs=3) as v3p,
            tc.tile_pool(name="rq", bufs=3) as rqp,
            tc.tile_pool(name="ev", bufs=3) as evp,
            tc.tile_pool(name="x0", bufs=3) as x0p,
            tc.tile_pool(name="v2", bufs=3) as v2p,
            tc.tile_pool(name="g", bufs=2) as gp,
            tc.tile_pool(name="st", bufs=2) as stp,
            tc.tile_pool(name="psum", bufs=4, space="PSUM") as pp,
        ):
            iota_t = cp.tile([P, P], F16)
            nc.sync.dma_start(out=iota_t[:], in_=iota_in[:])
            idx1_t = cp.tile([P, EDGE_COLS], I32)
            nc.sync.dma_start(out=idx1_t[:], in_=idx1_in[:])
            rowloc_t = cp.tile([P, EDGE_COLS], F16)
            nc.sync.dma_start(out=rowloc_t[:], in_=rowloc_in[:])
            degcol_t = cp.tile([P, EDGE_COLS], F32)
            nc.sync.dma_start(out=degcol_t[:], in_=degcol_in[:])
            degsh_t = cp.tile([P, NT], F32)
            nc.sync.dma_start(out=degsh_t[:], in_=degsh_in[:])
            fidx_loc_t = cp.tile([P, FCH], I32)
            nc.sync.dma_start(out=fidx_loc_t[:], in_=fidx_loc_in[:])
            fidx_mid_t = cp.tile([P, FCH], I32)
            nc.sync.dma_start(out=fidx_mid_t[:], in_=fidx_mid_in[:])

            def build_s(t):
                s_t = sp_.tile([P, NCHUNK, P], F16, tag="s")
                rl = rowloc_t[:, t * NCHUNK:(t + 1) * NCHUNK]
                rl_b = rl.to_broadcast([P, NCHUNK, P])
                io = iota_t[:]
                io_b = _dims(io, [io.ap[0], [0, NCHUNK], io.ap[1]])
                nc.vector.tensor_tensor(out=s_t[:], in0=rl_b, in1=io_b, op=eq)
                return s_t

            # whole-shard precomputes (hoisted out of the tile loops)
            # rq_all[:, 0, :] = rsqrt(deg_col) f16, rq_all[:, 1, :] = sqrt f16
            rq_all = cp.tile([P, 2, EDGE_COLS], F16)
            q32_all = cp.tile([P, EDGE_COLS], F32)
            nc.scalar.activation(q32_all[:], degcol_t[:], SQRT)
            nc.vector.tensor_copy(out=rq_all[:, 1, :], in_=q32_all[:])
            nc.vector.reciprocal(rq_all[:, 0, :], q32_all[:])
            # degrev_all [P, NT] f32; rq0_all [P, 2, NT] f32 (row scales)
            degrev_all = cp.tile([P, NT], F32)
            nc.vector.reciprocal(degrev_all[:], degsh_t[:])
            rq0_all = cp.tile([P, 2, NT], F32)
            nc.scalar.activation(rq0_all[:, 1, :], degsh_t[:], SQRT)
            nc.vector.reciprocal(rq0_all[:, 0, :], rq0_all[:, 1, :])
            # xs0 block 0 = x (DRAM->DRAM strided copy, once)
            x0dst = _dims(xs0_l[:, 0:D], [[192, NSH], [1, D]])
            nc.sync.dma_start(out=x0dst, in_=x_sh[:])

            # ================= hop 1 =================
            for t in range(NT if "hop1" not in ab else 0):
                v_t = v1p.tile([P, NCHUNK, D], F32, tag="v1")
                if t < 3 or "gather1" in ab:
                    nc.gpsimd.memset(v_t[:], 0.0)
                for j in range(NCHUNK if "gather1" not in ab else 0):
                    col = t * NCHUNK + j
                    nc.gpsimd.indirect_dma_start(
                        out=v_t[:, j, :], out_offset=None, in_=x_full[:],
                        in_offset=bass.IndirectOffsetOnAxis(
                            ap=idx1_t[:, col:col + 1], axis=0),
                        bounds_check=bnd_pad, oob_is_err=False,
                    )
                s_t = build_s(t)
                rq = rq_all[:, :, t * NCHUNK:(t + 1) * NCHUNK]
                # v3 [P, NCHUNK, 192] fp16 = [x | x*r | x*q] per chunk
                v3 = v3p.tile([P, NCHUNK, 192], F16, tag="v3")
                b0 = v3[:, :, 0:D]
                nc.scalar.activation(b0, v_t[:], COPY)
                b12 = _dims(v3[:, :, D:3 * D],
                            [v3[:].ap[0], [192, NCHUNK], [D, 2], [1, D]])
                v16b = _dims(v3[:, :, 0:D],
                             [v3[:].ap[0], [192, NCHUNK], [0, 2], [1, D]])
                rqb = _dims(rq, [rq_all[:].ap[0], [1, NCHUNK],
                                 [EDGE_COLS, 2], [0, D]])
                nc.vector.tensor_tensor(out=b12, in0=v16b, in1=rqb, op=mult)
                ps = pp.tile([P, 192], F32, space="PSUM", tag="ps")
                for j in range(NCHUNK):
                    nc.tensor.matmul(
                        out=ps[:], lhsT=s_t[:, j, :], rhs=v3[:, j, :],
                        start=(j == 0), stop=(j == NCHUNK - 1))
                y1_t = evp.tile([P, 192], F32, tag="y1")
                nc.scalar.activation(y1_t[:], ps[:], COPY,
                                     scale=degrev_all[:, t:t + 1])
                nc.sync.dma_start(out=y1_bounce[t * P:(t + 1) * P, :], in_=y1_t[:])
                # xs0 blocks 1-2 = x * {rsqrt(deg_row), sqrt(deg_row)}
                x_t = x0p.tile([P, D], F32, tag="xt")
                nc.sync.dma_start(out=x_t[:], in_=x_sh[t * P:(t + 1) * P, :])
                xs0_t = x0p.tile([P, 2, D], F32, tag="xs0")
                xb = _dims(x_t[:], [x_t[:].ap[0], [0, 2], [1, D]])
                rq0b = _dims(rq0_all[:, :, t:t + 1],
                             [rq0_all[:].ap[0], [NT, 2], [0, D]])
                nc.vector.tensor_tensor(out=xs0_t[:], in0=xb, in1=rq0b, op=mult)
                x12dst = _dims(xs0_l[t * P:(t + 1) * P, D:3 * D],
                               [[192, P], [1, 2 * D]])
                nc.sync.dma_start(out=x12dst, in_=xs0_t[:])

            # ================= AllGather =================
            if "ag" not in ab:
                nc.gpsimd.collective_compute(
                "AllGather", mybir.AluOpType.bypass,
                    replica_groups=[list(range(NC))],
                    ins=[y1_bounce[:]], outs=[y1full[:]],
                )

            # ================= hop 2 =================
            for t in range(NT if "hop2" not in ab else 0):
                v2 = v2p.tile([P, NCHUNK, 192], F32, tag="v2")
                if t < 3 or "gather2" in ab:
                    nc.gpsimd.memset(v2[:], 0.0)
                for j in range(NCHUNK if "gather2" not in ab else 0):
                    col = t * NCHUNK + j
                    nc.gpsimd.indirect_dma_start(
                        out=v2[:, j, :], out_offset=None, in_=y1full[:],
                        in_offset=bass.IndirectOffsetOnAxis(
                            ap=idx1_t[:, col:col + 1], axis=0),
                        bounds_check=bnd_pad, oob_is_err=False,
                    )
                s_t = build_s(t)
                v216 = v3p.tile([P, NCHUNK, 192], F16, tag="v216")
                nc.scalar.activation(v216[:], v2[:], COPY)
                ps = pp.tile([P, 192], F32, space="PSUM", tag="ps")
                for j in range(NCHUNK):
                    nc.tensor.matmul(
                        out=ps[:], lhsT=s_t[:, j, :], rhs=v216[:, j, :],
                        start=(j == 0), stop=(j == NCHUNK - 1))
                tmp = evp.tile([P, 192], F32, tag="tmp2")
                nc.scalar.activation(tmp[:], ps[:], COPY,
                                     scale=degrev_all[:, t:t + 1])
                xs0_t = x0p.tile([P, 192], F32, tag="xs0r")
                nc.sync.dma_start(out=xs0_t[:], in_=xs0_l[t * P:(t + 1) * P, :])
                xs2_t = evp.tile([P, 192], F32, tag="xs2")
                nc.vector.tensor_tensor(out=xs2_t[:], in0=tmp[:], in1=xs0_t[:], op=sub)
                nc.sync.dma_start(out=xs2_l[t * P:(t + 1) * P, :], in_=xs2_t[:])

            # ================= final gather + transpose =================
            tables = [xs0_l, y1full, xs2_l]
            fidx = [fidx_loc_t, fidx_mid_t, fidx_loc_t]
            bounds = [bnd_sh, bnd_pad, bnd_sh]
            for pc_i in range(FCH // PC if "final" not in ab else 0):
                gs = []
                for h in range(3):
                    g = gp.tile([P, PC, 192], F32, tag=f"g{h}")
                    if pc_i < 2 or "gatherf" in ab:
                        nc.gpsimd.memset(g[:], 0.0)
                    for j in range(PC if "gatherf" not in ab else 0):
                        col = pc_i * PC + j
                        nc.gpsimd.indirect_dma_start(
                            out=g[:, j, :], out_offset=None, in_=tables[h][:],
                            in_offset=bass.IndirectOffsetOnAxis(
                                ap=fidx[h][:, col:col + 1], axis=0),
                            bounds_check=bounds[h], oob_is_err=False,
                        )
                    gs.append(g)
                stage = stp.tile([P, PC, D * 9], F32, tag="stage")
                for k in range(9):
                    h, b = divmod(k, 3)
                    src = gs[h][:, :, b * D:(b + 1) * D]
                    dst = _dims(stage[:, :, k:k + 1],
                                [stage[:].ap[0], [D * 9, PC], [9, D]])
                    if k % 2 == 0:
                        nc.vector.tensor_copy(out=dst, in_=src)
                    else:
                        nc.scalar.activation(dst, src, COPY)
                obase = out_f[pc_i * PC * P:(pc_i + 1) * PC * P, :]
                orows = _dims(obase, [[576, P], [P * 576, PC], [1, 576]])
                nc.sync.dma_start(out=orows, in_=stage[:])

    _split_multi_waits(nc)
    return nc


def _plan(x, deg, adj_row, adj_col, edge):
    """Host-side sharding: pure index bookkeeping + input reordering."""
    x = np.asarray(x, np.float32)
    deg = np.asarray(deg, np.float32).reshape(-1)
    adj_row = np.asarray(adj_row, np.int64)
    adj_col = np.asarray(adj_col, np.int64)
    edge = np.asarray(edge, np.int64)

    x_full = np.zeros((NPAD, D), np.float32)
    x_full[:N] = x
    iota_np = np.tile(np.arange(P, dtype=np.float16), (P, 1))
    ep = edge.reshape(-1)

    in_maps, positions = [], []
    for c in range(NC):
        r0 = c * NSH
        idx1 = np.full((P, EDGE_COLS), BIG, np.int32)
        rowloc = np.full((P, EDGE_COLS), -1.0, np.float16)
        degcol = np.ones((P, EDGE_COLS), np.float32)
        for t in range(NT):
            base = r0 + t * P
            lo = np.searchsorted(adj_row, base, side="left")
            hi = np.searchsorted(adj_row, base + P, side="left")
            n_e = hi - lo
            assert n_e <= NCHUNK * P, f"tile overflow: {n_e}"
            sl = np.arange(n_e)
            jj, pp_ = divmod(sl, P)
            colbase = t * NCHUNK
            idx1[pp_, colbase + jj] = adj_col[lo:hi]
            rowloc[pp_, colbase + jj] = (adj_row[lo:hi] - base).astype(np.float16)
            degcol[pp_, colbase + jj] = deg[adj_col[lo:hi]]
        real = min(NSH, max(0, N - r0))
        dlocal = np.ones(NSH, np.float32)
        dlocal[:real] = deg[r0:r0 + real]
        degsh = dlocal.reshape(NT, P).T.copy()

        x_shard = np.zeros((NSH, D), np.float32)
        x_shard[:real] = x[r0:r0 + real]

        mine = np.nonzero((ep >= r0) & (ep < r0 + NSH))[0]
        n_c = len(mine)
        assert n_c <= FCH * P, f"endpoint overflow: {n_c}"
        fidx_loc = np.full((P, FCH), BIG, np.int32)
        fidx_mid = np.full((P, FCH), BIG, np.int32)
        sl = np.arange(n_c)
        jj, pp_ = divmod(sl, P)
        fidx_loc[pp_, jj] = (ep[mine] - r0).astype(np.int32)
        fidx_mid[pp_, jj] = ep[mine].astype(np.int32)
        positions.append(mine)

        in_maps.append({
            "x_full": x_full,
            "x_sh": x_shard,
            "degsh": degsh,
            "idx1": idx1,
            "rowloc": rowloc,
            "degcol": degcol,
            "fidx_loc": fidx_loc,
            "fidx_mid": fidx_mid,
            "iota": iota_np,
        })
    return in_maps, positions


def _assemble(results, positions):
    out = np.zeros((2 * EQ, 576), np.float32)
    for c in range(NC):
        rows = results[c]["out_f"]
        n_c = len(positions[c])
        out[positions[c]] = rows[:n_c]
    return out.reshape(2, EQ, D, 9)


def kernel(x, deg, adj_row, adj_col, edge):
    import time
    if "nc" not in _prog_cache:
        t0 = time.time()
        _prog_cache["nc"] = _build_program()
        print(f"[kernel] program build: {time.time()-t0:.1f}s", flush=True)
    nc = _prog_cache["nc"]
    t0 = time.time()
    in_maps, positions = _plan(x, deg, adj_row, adj_col, edge)
    print(f"[kernel] host plan: {time.time()-t0:.1f}s", flush=True)
    t0 = time.time()
    res = run_bass_kernel_spmd(nc, in_maps, list(range(NC)))
    print(f"[kernel] compile+run: {time.time()-t0:.1f}s", flush=True)
    return _assemble(res.results, positions)



# revision 8
# speedup vs baseline: 1.5759x; 1.5759x over previous
"""Distributed Trainium2 Bass kernel for nn_NodeFeat (2-hop Chebyshev-style GNN
feature expansion + edge gather), 8 NeuronCores.

v2: batched SWDGE gathers (InstDMAGatherAnt) replace per-chunk indirect DMAs.

Node sharding per the problem's sharding hint:
  - 50000 nodes padded to 50176 = 8 x 6272; core c owns natural rows
    [6272c, 6272c+6272). Gather tables (x_full, y1full) use a permuted
    "gid" numbering gid = g*7168 + c*896 + r (g = AllGather group 0..6,
    r = row within the core's group slice) so that group-wise AllGathers
    land contiguously and one index space serves both hops.
  - Edges are packed per 128-row tile into 18 chunks of 128 slots:
    9 "lo" chunks (col gid < 25088) + 9 "hi" chunks (col gid >= 25088);
    the lo/hi split keeps dma_gather's int16 indices in range via a
    table base offset. Pad slots use idx 0 with selector rowloc=-1.
  - hop1: one dma_gather pair (8064 idxs) per 7-tile group fetches x[col]
    rows f32; per-edge scaling {1, rsqrt(deg_col), sqrt(deg_col)} builds a
    [128,18,192] fp16 moving operand; segment-sum on TensorE via one-hot
    selector matmuls accumulated in PSUM; ScalarE evacuates with the
    1/deg row scale to fp16.
  - y1 stored fp16 in 256-wide rows (512B, dma_gather elem granularity);
    7 group-wise AllGathers overlap with hop1 compute.
  - hop2: dma_gather pairs fetch y1full rows fp16 (direct moving operand,
    no conversion); same selectors; minus xs0 (kept resident in SBUF).
  - final: fused table h_l [6272, 640] fp16 = [xs0|y1|xs2|pad] per local
    row; endpoints partitioned by owner core; 6 piece gathers + f32
    convert + packed row writes. Host reorders/transposes (bookkeeping).

All floating-point math runs on device; the host only shards, pads,
reorders and reassembles (index bookkeeping).
"""
import numpy as np

import concourse.bass as bass
import concourse.mybir as mybir
import concourse.tile as tile
from concourse import library_config
from concourse.bass_utils import run_bass_kernel_spmd

# ---------------- hardcoded problem geometry ----------------
N = 50000
D = 64
EQ = 32768
P = 128
NC = 8                   # cores
NT = 49                  # row tiles per core
NSH = NT * P             # 6272 rows per core
NPAD = NSH * NC          # 50176
NG = 7                   # AllGather groups
GT = 7                   # tiles per group
GR = GT * P              # 896 rows per group per core
NL = 9                   # lo chunks per tile
NH = 9                   # hi chunks per tile
NCHUNK = NL + NH         # 18
LOCUT = NPAD // 2        # 25088: idx < LOCUT gathers from base table
TILE_SLOTS = NL * P      # 1152 slots per tile per side
GSLOTS = 2 * GT * TILE_SLOTS   # 16128 slots per group (lo then hi)
STREAM = NG * GSLOTS     # 112896 slots per core
FCH = 66                 # final-gather chunks (66*128 = 8448 slots)
FPC = 6                  # final pieces
FCPP = FCH // FPC        # 11 chunks per piece
HW = 640                 # h_l row width (576 used + 64 pad, 1280B)
YW = 256                 # y1 row width fp16 (192 used + 64 pad, 512B)
F32 = mybir.dt.float32
F16 = mybir.dt.float16
I16 = mybir.dt.int16

_prog_cache = {}


class _TC(tile.TileContext):
    """TileContext whose final drain splits sem waits one-per-instruction
    (this walrus rejects >1 sync wait on an instruction)."""

    def _drain_and_barrier(self, tick_clock, wait_clock):
        nc = self.nc
        probe = nc.sync.nop()
        wait_clock.add_sem_waits(
            probe.ins, tile.ScopedClock({None: tick_clock.global_clock}))
        si = probe.ins.sync_info
        waits = list(si.on_wait) if si and si.on_wait else []
        if si is not None:
            si.on_wait = waits[:1]
        for w in waits[1:]:
            n2 = nc.sync.nop()
            if n2.ins.sync_info is None:
                n2.ins.sync_info = mybir.SyncInfo(on_wait=[w], on_update=[])
            else:
                n2.ins.sync_info.on_wait = [w]
        nc.sync.drain()
        nc.all_engine_barrier()
        popped = nc._tile_sem_poison_stack.pop()
        assert popped is self._sem_poison
        nc.clear_and_free_semaphores(list(self.sems.allocated().values()))
        nc.all_engine_barrier()


def _split_multi_waits(nc):
    for fn in nc.m.functions:
        for blk in fn.blocks:
            new_list = []
            for inst in blk.instructions:
                si = inst.sync_info
                waits = list(si.on_wait) if si and si.on_wait else []
                if len(waits) > 1:
                    for j, w in enumerate(waits[:-1]):
                        nop = mybir.InstNoOp(
                            name=f"{inst.name}-ws{j}",
                            engine=inst.engine,
                            ins=[], outs=[],
                            sync_info=mybir.SyncInfo(on_wait=[w], on_update=[]),
                        )
                        nc.register_instruction(nop, overwrite=True)
                        new_list.append(nop)
                    si.on_wait = waits[-1:]
                new_list.append(inst)
            blk.instructions[:] = new_list


def _dims(ap, dims):
    """Same tensor+offset as `ap`, explicit [stride(elem), nelem] dims."""
    return bass.AP(ap.tensor, ap.offset, dims)


def _build_program(ablate=()):
    """ablate: subset of {"hop1","ag","hop2","final"} to SKIP (perf
    ablation only -- results become wrong)."""
    ab = set(ablate)
    nc = bass.Bass("TRN2", target_bir_lowering=False, debug=False, num_devices=NC)

    x_full = nc.dram_tensor("x_full", [NPAD, D], F32, kind="ExternalInput")
    x_sh = nc.dram_tensor("x_sh", [NSH, D], F32, kind="ExternalInput")
    degsh_in = nc.dram_tensor("degsh", [P, NT], F32, kind="ExternalInput")
    idx1_in = nc.dram_tensor("idx1", [P, STREAM // 16], I16, kind="ExternalInput")
    fidx_in = nc.dram_tensor("fidx", [P, FCH * 8], I16, kind="ExternalInput")
    rowloc_in = nc.dram_tensor("rowloc", [P, NT * NCHUNK], F16, kind="ExternalInput")
    degcol_in = nc.dram_tensor("degcol", [P, NT * NCHUNK], F32, kind="ExternalInput")
    iota_in = nc.dram_tensor("iota", [P, P], F16, kind="ExternalInput")

    out_f = nc.dram_tensor("out_f", [FCH * P, 576], F32, kind="ExternalOutput")

    y1_bounce = nc.dram_tensor("y1_bounce", [NSH, YW], F16)
    y1full = nc.dram_tensor("y1full", [NPAD, YW], F16, addr_space="Shared")
    h_l = nc.dram_tensor("h_l", [NSH, HW], F16)

    eq = mybir.AluOpType.is_equal
    mult = mybir.AluOpType.mult
    sub = mybir.AluOpType.subtract
    COPY = mybir.ActivationFunctionType.Copy
    SQRT = mybir.ActivationFunctionType.Sqrt
    EC = NT * NCHUNK  # 882 selector columns

    with _TC(nc) as tc, nc.allow_low_precision(
            reason="fp16 tables + matmul operands; PSUM accumulates in f32"):
        nc.gpsimd.load_library(library_config.mlp)
        with (
            tc.tile_pool(name="const", bufs=1) as cp,
            tc.tile_pool(name="s", bufs=3) as sp_,
            tc.tile_pool(name="ev", bufs=3) as evp,
            tc.tile_pool(name="x0", bufs=3) as x0p,
            tc.tile_pool(name="psum", bufs=4, space="PSUM") as pp,
        ):
            iota_t = cp.tile([P, P], F16)
            nc.sync.dma_start(out=iota_t[:], in_=iota_in[:])
            idx1_t = cp.tile([P, STREAM // 16], I16)
            nc.sync.dma_start(out=idx1_t[:], in_=idx1_in[:])
            fidx_t = cp.tile([P, FCH * 8], I16)
            nc.sync.dma_start(out=fidx_t[:], in_=fidx_in[:])
            rowloc_t = cp.tile([P, EC], F16)
            nc.sync.dma_start(out=rowloc_t[:], in_=rowloc_in[:])
            degcol_t = cp.tile([P, EC], F32)
            nc.sync.dma_start(out=degcol_t[:], in_=degcol_in[:])
            degsh_t = cp.tile([P, NT], F32)
            nc.sync.dma_start(out=degsh_t[:], in_=degsh_in[:])
            xs0_sb = cp.tile([P, NT, 192], F16)
            # zero-fill the pad columns of y1_bounce / h_l once
            zt = cp.tile([P, NT, D], F16)
            nc.gpsimd.memset(zt[:], 0.0)
            zdst1 = _dims(y1_bounce[:, 192:YW], [[YW, NSH], [1, YW - 192]])
            nc.sync.dma_start(out=zdst1, in_=zt[:])
            zdst2 = _dims(h_l[:, 576:HW], [[HW, NSH], [1, HW - 576]])
            nc.sync.dma_start(out=zdst2, in_=zt[:])

            def build_s(t):
                s_t = sp_.tile([P, NCHUNK, P], F16, tag="s")
                rl = rowloc_t[:, t * NCHUNK:(t + 1) * NCHUNK]
                rl_b = rl.to_broadcast([P, NCHUNK, P])
                io = iota_t[:]
                io_b = _dims(io, [io.ap[0], [0, NCHUNK], io.ap[1]])
                nc.vector.tensor_tensor(out=s_t[:], in0=rl_b, in1=io_b, op=eq)
                return s_t

            # whole-shard precomputes
            rq_all = cp.tile([P, 2, EC], F16)
            q32_all = cp.tile([P, EC], F32)
            nc.scalar.activation(q32_all[:], degcol_t[:], SQRT)
            nc.vector.tensor_copy(out=rq_all[:, 1, :], in_=q32_all[:])
            nc.vector.reciprocal(rq_all[:, 0, :], q32_all[:])
            degrev_all = cp.tile([P, NT], F32)
            nc.vector.reciprocal(degrev_all[:], degsh_t[:])
            rq0_all = cp.tile([P, 2, NT], F32)
            nc.scalar.activation(rq0_all[:, 1, :], degsh_t[:], SQRT)
            nc.vector.reciprocal(rq0_all[:, 0, :], rq0_all[:, 1, :])

            # ================= hop 1 =================
            with tc.tile_pool(name="v1", bufs=2) as v1p, \
                    tc.tile_pool(name="v3", bufs=3) as v3p:

                def h1_gather(g):
                    v = v1p.tile([P, 2 * GT * NL, D], F32, tag="v1")
                    b = g * GSLOTS
                    half = GT * TILE_SLOTS
                    nc.gpsimd.dma_gather(
                        v[:, 0:GT * NL, :], x_full[:],
                        idx1_t[:, b // 16:(b + half) // 16], half, half, D,
                        single_packet=False)
                    nc.gpsimd.dma_gather(
                        v[:, GT * NL:, :], x_full[LOCUT:, :],
                        idx1_t[:, (b + half) // 16:(b + 2 * half) // 16],
                        half, half, D, single_packet=False)
                    return v

                vq = {}
                if "hop1" not in ab:
                    vq[0] = h1_gather(0)
                for g in range(NG if "hop1" not in ab else 0):
                    if g + 1 < NG:
                        vq[g + 1] = h1_gather(g + 1)
                    v = vq.pop(g)
                    for tau in range(GT):
                        t = GT * g + tau
                        s_t = build_s(t)
                        v3 = v3p.tile([P, NCHUNK, 192], F16, tag="v3")
                        # b0: gathered x values (lo block, hi block)
                        nc.scalar.activation(
                            v3[:, 0:NL, 0:D], v[:, NL * tau:NL * (tau + 1), :],
                            COPY)
                        nc.scalar.activation(
                            v3[:, NL:NCHUNK, 0:D],
                            v[:, GT * NL + NL * tau:GT * NL + NL * (tau + 1), :],
                            COPY)
                        # b12: x * {rsqrt(deg_col), sqrt(deg_col)} per half
                        for h in range(2):
                            dst = _dims(v3[:, h * NL:(h + 1) * NL, D:3 * D],
                                        [v3[:].ap[0], [192, NL], [D, 2], [1, D]])
                            src = _dims(v3[:, h * NL:(h + 1) * NL, 0:D],
                                        [v3[:].ap[0], [192, NL], [0, 2], [1, D]])
                            rq = rq_all[:, :, t * NCHUNK + h * NL:
                                        t * NCHUNK + (h + 1) * NL]
                            rqb = _dims(rq, [rq_all[:].ap[0], [1, NL],
                                             [EC, 2], [0, D]])
                            nc.vector.tensor_tensor(out=dst, in0=src, in1=rqb,
                                                    op=mult)
                        ps = pp.tile([P, 192], F32, space="PSUM", tag="ps")
                        for j in range(NCHUNK):
                            cc = NL * tau + j % NL + (GT * NL if j >= NL else 0)
                            nc.tensor.matmul(
                                out=ps[:], lhsT=s_t[:, j, :], rhs=v3[:, j, :],
                                start=(j == 0), stop=(j == NCHUNK - 1))
                        y1_t = evp.tile([P, 192], F16, tag="y1")
                        nc.scalar.activation(y1_t[:], ps[:], COPY,
                                             scale=degrev_all[:, t:t + 1])
                        ydst = _dims(y1_bounce[t * P:(t + 1) * P, 0:192],
                                     [[YW, P], [1, 192]])
                        nc.sync.dma_start(out=ydst, in_=y1_t[:])
                        hdst = _dims(h_l[t * P:(t + 1) * P, 192:384],
                                     [[HW, P], [1, 192]])
                        nc.sync.dma_start(out=hdst, in_=y1_t[:])
                        # xs0 = [x | x*rsqrt(deg_row) | x*sqrt(deg_row)]
                        x_t = x0p.tile([P, D], F32, tag="xt")
                        nc.sync.dma_start(out=x_t[:],
                                          in_=x_sh[t * P:(t + 1) * P, :])
                        nc.scalar.activation(xs0_sb[:, t, 0:D], x_t[:], COPY)
                        xb = _dims(x_t[:], [x_t[:].ap[0], [0, 2], [1, D]])
                        rq0b = _dims(rq0_all[:, :, t:t + 1],
                                     [rq0_all[:].ap[0], [NT, 2], [0, D]])
                        x12 = _dims(xs0_sb[:, t, D:3 * D],
                                    [xs0_sb[:].ap[0], [D, 2], [1, D]])
                        nc.vector.tensor_tensor(out=x12, in0=xb, in1=rq0b,
                                                op=mult)
                        h0dst = _dims(h_l[t * P:(t + 1) * P, 0:192],
                                      [[HW, P], [1, 192]])
                        nc.sync.dma_start(out=h0dst, in_=xs0_sb[:, t, :])
                    if "ag" not in ab:
                        nc.gpsimd.collective_compute(
                            "AllGather", mybir.AluOpType.bypass,
                            replica_groups=[list(range(NC))],
                            ins=[y1_bounce[g * GR:(g + 1) * GR, :]],
                            outs=[y1full[g * NC * GR:(g + 1) * NC * GR, :]],
                        )

            # ================= hop 2 =================
            # subgroups of (4, 3) tiles; v2 holds lo+hi chunks contiguously
            SUBS = [(0, 4), (4, 7)]
            with tc.tile_pool(name="v2", bufs=2) as v2p:

                def h2_gather(g, si):
                    t0, t1 = SUBS[si]
                    nch = NL * (t1 - t0)
                    v2 = v2p.tile([P, 2 * NL * 4, YW], F16, tag="v2")
                    b = g * GSLOTS
                    lo0 = b + t0 * TILE_SLOTS
                    hi0 = b + GT * TILE_SLOTS + t0 * TILE_SLOTS
                    n = nch * P
                    nc.gpsimd.dma_gather(
                        v2[:, 0:nch, :], y1full[:],
                        idx1_t[:, lo0 // 16:(lo0 + n) // 16], n, n, YW,
                        single_packet=False)
                    nc.gpsimd.dma_gather(
                        v2[:, nch:2 * nch, :], y1full[LOCUT:, :],
                        idx1_t[:, hi0 // 16:(hi0 + n) // 16], n, n, YW,
                        single_packet=False)
                    return v2

                pairs = [(g, si) for g in range(NG) for si in range(2)]
                v2q = {}
                if "hop2" not in ab:
                    v2q[pairs[0]] = h2_gather(*pairs[0])
                for pi, (g, si) in enumerate(pairs if "hop2" not in ab else []):
                    if pi + 1 < len(pairs):
                        v2q[pairs[pi + 1]] = h2_gather(*pairs[pi + 1])
                    v2 = v2q.pop((g, si))
                    t0, t1 = SUBS[si]
                    nch = NL * (t1 - t0)
                    for tau in range(t0, t1):
                        t = GT * g + tau
                        s_t = build_s(t)
                        ps = pp.tile([P, 192], F32, space="PSUM", tag="ps")
                        for j in range(NCHUNK):
                            cc = NL * (tau - t0) + j % NL + (nch if j >= NL else 0)
                            nc.tensor.matmul(
                                out=ps[:], lhsT=s_t[:, j, :],
                                rhs=v2[:, cc, 0:192],
                                start=(j == 0), stop=(j == NCHUNK - 1))
                        tmp = evp.tile([P, 192], F16, tag="tmp2")
                        nc.scalar.activation(tmp[:], ps[:], COPY,
                                             scale=degrev_all[:, t:t + 1])
                        xs2 = evp.tile([P, 192], F16, tag="xs2")
                        nc.vector.tensor_tensor(out=xs2[:], in0=tmp[:],
                                                in1=xs0_sb[:, t, :], op=sub)
                        h2dst = _dims(h_l[t * P:(t + 1) * P, 384:576],
                                      [[HW, P], [1, 192]])
                        nc.sync.dma_start(out=h2dst, in_=xs2[:])

            # ================= final gather =================
            with tc.tile_pool(name="g", bufs=2) as gp, \
                    tc.tile_pool(name="st", bufs=2) as stp:
                for p_i in range(FPC if "final" not in ab else 0):
                    gth = gp.tile([P, FCPP, HW], F16, tag="g")
                    n = FCPP * P
                    c0 = p_i * FCPP * 8
                    nc.gpsimd.dma_gather(
                        gth[:], h_l[:], fidx_t[:, c0:c0 + FCPP * 8], n, n, HW,
                        single_packet=False)
                    stage = stp.tile([P, FCPP, 576], F32, tag="st")
                    if p_i % 2 == 0:
                        nc.vector.tensor_copy(out=stage[:],
                                              in_=gth[:, :, 0:576])
                    else:
                        nc.scalar.activation(stage[:], gth[:, :, 0:576], COPY)
                    obase = out_f[p_i * FCPP * P:(p_i + 1) * FCPP * P, :]
                    orows = _dims(obase, [[576, P], [P * 576, FCPP], [1, 576]])
                    nc.sync.dma_start(out=orows, in_=stage[:])

    _split_multi_waits(nc)
    mybir.codegen_inst_isa_subclasses(nc)
    return nc


def _plan(x, deg, adj_row, adj_col, edge):
    """Host-side sharding: pure index bookkeeping + input reordering."""
    x = np.asarray(x, np.float32)
    deg = np.asarray(deg, np.float32).reshape(-1)
    adj_row = np.asarray(adj_row, np.int64)
    adj_col = np.asarray(adj_col, np.int64)
    edge = np.asarray(edge, np.int64)
    nnz = len(adj_row)

    # natural n -> gid = g*7168 + c*896 + r
    n_ids = np.arange(NPAD)
    c_of = n_ids // NSH
    loc_of = n_ids % NSH
    g_of = loc_of // GR
    r_of = loc_of % GR
    gid_of = g_of * (NC * GR) + c_of * GR + r_of

    x_full = np.zeros((NPAD, D), np.float32)
    x_full[gid_of[:N]] = x
    degp = np.ones(NPAD, np.float32)
    degp[:N] = deg
    iota_np = np.tile(np.arange(P, dtype=np.float16), (P, 1))

    # edge -> slot assignment (vectorized, global)
    t_glob = adj_row // P               # 0..391
    lane = (adj_row % P).astype(np.float16)
    gidc = gid_of[adj_col]
    is_hi = gidc >= LOCUT
    grp = t_glob * 2 + is_hi            # (tile, side) group id
    order = np.argsort(grp, kind="stable")
    counts = np.bincount(grp, minlength=NT * NC * 2)
    starts = np.concatenate(([0], np.cumsum(counts)))[:-1]
    rank = np.empty(nnz, np.int64)
    rank[order] = np.arange(nnz) - starts[grp[order]]
    assert counts.max() <= TILE_SLOTS, f"tile side overflow: {counts.max()}"

    core_e = t_glob // NT
    tcore = t_glob % NT
    g_e = tcore // GT
    tau_e = tcore % GT
    # stream position within the core
    pos = (g_e * GSLOTS + is_hi * (GT * TILE_SLOTS) + tau_e * TILE_SLOTS
           + rank)
    idx_val = (gidc - is_hi * LOCUT).astype(np.int16)
    # selector columns (tile-major)
    sel_col = tcore * NCHUNK + is_hi * NL + rank // P
    sel_p = rank % P

    def wrap(stream):
        w = stream.reshape(-1, 16).T.copy()
        return np.tile(w, (NC, 1))

    ep = edge.reshape(-1)
    ep_core = ep // NSH
    ep_loc = ep % NSH

    in_maps, positions = [], []
    for c in range(NC):
        m = core_e == c
        s1 = np.zeros(STREAM, np.int16)
        s1[pos[m]] = idx_val[m]
        rowloc = np.full((P, NT * NCHUNK), -1.0, np.float16)
        rowloc[sel_p[m], sel_col[m]] = lane[m]
        degcol = np.ones((P, NT * NCHUNK), np.float32)
        degcol[sel_p[m], sel_col[m]] = degp[adj_col[m]]

        dlocal = degp[c * NSH:(c + 1) * NSH]
        degsh = dlocal.reshape(NT, P).T.copy()
        x_shard = np.zeros((NSH, D), np.float32)
        real = min(NSH, max(0, N - c * NSH))
        x_shard[:real] = x[c * NSH:c * NSH + real]

        mine = np.nonzero(ep_core == c)[0]
        n_c = len(mine)
        assert n_c <= FCH * P, f"endpoint overflow: {n_c}"
        fs = np.zeros(FCH * P, np.int16)
        fs[:n_c] = ep_loc[mine].astype(np.int16)
        positions.append(mine)

        in_maps.append({
            "x_full": x_full,
            "x_sh": x_shard,
            "degsh": degsh,
            "idx1": wrap(s1),
            "fidx": wrap(fs),
            "rowloc": rowloc,
            "degcol": degcol,
            "iota": iota_np,
        })
    return in_maps, positions


def _assemble(results, positions):
    out = np.zeros((2 * EQ, 576), np.float32)
    for c in range(NC):
        rows = results[c]["out_f"]
        n_c = len(positions[c])
        out[positions[c]] = rows[:n_c]
    return out.reshape(2, EQ, 9, D).transpose(0, 1, 3, 2)


def kernel(x, deg, adj_row, adj_col, edge):
    import time
    if "nc" not in _prog_cache:
        t0 = time.time()
        _prog_cache["nc"] = _build_program()
        print(f"[kernel] program build: {time.time()-t0:.1f}s", flush=True)
    nc = _prog_cache["nc"]
    t0 = time.time()
    in_maps, positions = _plan(x, deg, adj_row, adj_col, edge)
    print(f"[kernel] host plan: {time.time()-t0:.1f}s", flush=True)
    t0 = time.time()
    res = run_bass_kernel_spmd(nc, in_maps, list(range(NC)))
    print(f"[kernel] compile+run: {time.time()-t0:.1f}s", flush=True)
    return _assemble(res.results, positions)


# revision 9
# speedup vs baseline: 2.7965x; 1.7745x over previous
"""Distributed Trainium2 Bass kernel for nn_NodeFeat (2-hop Chebyshev-style GNN
feature expansion + edge gather), 8 NeuronCores.

v2: batched SWDGE gathers (InstDMAGatherAnt) replace per-chunk indirect DMAs.

Node sharding per the problem's sharding hint:
  - 50000 nodes padded to 50176 = 8 x 6272; core c owns natural rows
    [6272c, 6272c+6272). Gather tables (x_full, y1full) use a permuted
    "gid" numbering gid = g*7168 + c*896 + r (g = AllGather group 0..6,
    r = row within the core's group slice) so that group-wise AllGathers
    land contiguously and one index space serves both hops.
  - Edges are packed per 128-row tile into 18 chunks of 128 slots:
    9 "lo" chunks (col gid < 25088) + 9 "hi" chunks (col gid >= 25088);
    the lo/hi split keeps dma_gather's int16 indices in range via a
    table base offset. Pad slots use idx 0 with selector rowloc=-1.
  - hop1: one dma_gather pair (8064 idxs) per 7-tile group fetches x[col]
    rows f32; per-edge scaling {1, rsqrt(deg_col), sqrt(deg_col)} builds a
    [128,18,192] fp16 moving operand; segment-sum on TensorE via one-hot
    selector matmuls accumulated in PSUM; ScalarE evacuates with the
    1/deg row scale to fp16.
  - y1 stored fp16 in 256-wide rows (512B, dma_gather elem granularity);
    7 group-wise AllGathers overlap with hop1 compute.
  - hop2: dma_gather pairs fetch y1full rows fp16 (direct moving operand,
    no conversion); same selectors; minus xs0 (kept resident in SBUF).
  - final: fused table h_l [6272, 640] fp16 = [xs0|y1|xs2|pad] per local
    row; endpoints partitioned by owner core; 6 piece gathers + f32
    convert + packed row writes. Host reorders/transposes (bookkeeping).

All floating-point math runs on device; the host only shards, pads,
reorders and reassembles (index bookkeeping).
"""
import numpy as np

import concourse.bass as bass
import concourse.mybir as mybir
import concourse.tile as tile
from concourse import library_config
from concourse.bass_utils import run_bass_kernel_spmd

# ---------------- hardcoded problem geometry ----------------
N = 50000
D = 64
EQ = 32768
P = 128
NC = 8                   # cores
NT = 49                  # row tiles per core
NSH = NT * P             # 6272 rows per core
NPAD = NSH * NC          # 50176
NG = 7                   # AllGather groups
GT = 7                   # tiles per group
GR = GT * P              # 896 rows per group per core
NL = 9                   # lo chunks per tile
NH = 9                   # hi chunks per tile
NCHUNK = NL + NH         # 18
LOCUT = NPAD // 2        # 25088: idx < LOCUT gathers from base table
TILE_SLOTS = NL * P      # 1152 slots per tile per side
GSLOTS = 2 * GT * TILE_SLOTS   # 16128 slots per group (lo then hi)
STREAM = NG * GSLOTS     # 112896 slots per core
FCH = 66                 # final-gather chunks (66*128 = 8448 slots)
FPC = 6                  # final pieces
FCPP = FCH // FPC        # 11 chunks per piece
HW = 640                 # h_l row width (576 used + 64 pad, 1280B)
YW = 256                 # y1 row width fp16 (192 used + 64 pad, 512B)
F32 = mybir.dt.float32
F16 = mybir.dt.float16
I16 = mybir.dt.int16

_prog_cache = {}


class _TC(tile.TileContext):
    """TileContext whose final drain splits sem waits one-per-instruction
    (this walrus rejects >1 sync wait on an instruction)."""

    def _drain_and_barrier(self, tick_clock, wait_clock):
        nc = self.nc
        probe = nc.sync.nop()
        wait_clock.add_sem_waits(
            probe.ins, tile.ScopedClock({None: tick_clock.global_clock}))
        si = probe.ins.sync_info
        waits = list(si.on_wait) if si and si.on_wait else []
        if si is not None:
            si.on_wait = waits[:1]
        for w in waits[1:]:
            n2 = nc.sync.nop()
            if n2.ins.sync_info is None:
                n2.ins.sync_info = mybir.SyncInfo(on_wait=[w], on_update=[])
            else:
                n2.ins.sync_info.on_wait = [w]
        nc.sync.drain()
        nc.all_engine_barrier()
        popped = nc._tile_sem_poison_stack.pop()
        assert popped is self._sem_poison
        nc.clear_and_free_semaphores(list(self.sems.allocated().values()))
        nc.all_engine_barrier()


def _split_multi_waits(nc):
    for fn in nc.m.functions:
        for blk in fn.blocks:
            new_list = []
            for inst in blk.instructions:
                si = inst.sync_info
                waits = list(si.on_wait) if si and si.on_wait else []
                if len(waits) > 1:
                    for j, w in enumerate(waits[:-1]):
                        nop = mybir.InstNoOp(
                            name=f"{inst.name}-ws{j}",
                            engine=inst.engine,
                            ins=[], outs=[],
                            sync_info=mybir.SyncInfo(on_wait=[w], on_update=[]),
                        )
                        nc.register_instruction(nop, overwrite=True)
                        new_list.append(nop)
                    si.on_wait = waits[-1:]
                new_list.append(inst)
            blk.instructions[:] = new_list


def _dims(ap, dims):
    """Same tensor+offset as `ap`, explicit [stride(elem), nelem] dims."""
    return bass.AP(ap.tensor, ap.offset, dims)


def _build_program(ablate=()):
    """ablate: subset of {"hop1","ag","hop2","final"} to SKIP (perf
    ablation only -- results become wrong)."""
    ab = set(ablate)
    nc = bass.Bass("TRN2", target_bir_lowering=False, debug=False, num_devices=NC,
                   num_swdge_queues=4)

    x_full = nc.dram_tensor("x_full", [NPAD, D], F32, kind="ExternalInput")
    x_sh = nc.dram_tensor("x_sh", [NSH, D], F32, kind="ExternalInput")
    degsh_in = nc.dram_tensor("degsh", [P, NT], F32, kind="ExternalInput")
    idx1_in = nc.dram_tensor("idx1", [P, STREAM // 16], I16, kind="ExternalInput")
    fidx_in = nc.dram_tensor("fidx", [P, FCH * 8], I16, kind="ExternalInput")
    rowloc_in = nc.dram_tensor("rowloc", [P, NT * NCHUNK], F16, kind="ExternalInput")
    degcol_in = nc.dram_tensor("degcol", [P, NT * NCHUNK], F32, kind="ExternalInput")
    iota_in = nc.dram_tensor("iota", [P, P], F16, kind="ExternalInput")

    out_f = nc.dram_tensor("out_f", [FCH * P, 576], F32, kind="ExternalOutput")

    y1_bounce = nc.dram_tensor("y1_bounce", [NSH, YW], F16)
    y1full = nc.dram_tensor("y1full", [NPAD, YW], F16, addr_space="Shared")
    h_l = nc.dram_tensor("h_l", [NSH, HW], F16)

    eq = mybir.AluOpType.is_equal
    mult = mybir.AluOpType.mult
    sub = mybir.AluOpType.subtract
    COPY = mybir.ActivationFunctionType.Copy
    SQRT = mybir.ActivationFunctionType.Sqrt
    EC = NT * NCHUNK  # 882 selector columns

    qrr = [0]

    def _q():
        qrr[0] = (qrr[0] + 1) % 4
        return qrr[0]

    with _TC(nc) as tc, nc.allow_low_precision(
            reason="fp16 tables + matmul operands; PSUM accumulates in f32"):
        nc.gpsimd.load_library(library_config.mlp)
        with (
            tc.tile_pool(name="const", bufs=1) as cp,
            tc.tile_pool(name="s", bufs=3) as sp_,
            tc.tile_pool(name="ev", bufs=3) as evp,
            tc.tile_pool(name="x0", bufs=3) as x0p,
            tc.tile_pool(name="psum", bufs=4, space="PSUM") as pp,
        ):
            iota_t = cp.tile([P, P], F16)
            nc.sync.dma_start(out=iota_t[:], in_=iota_in[:])
            idx1_t = cp.tile([P, STREAM // 16], I16)
            nc.sync.dma_start(out=idx1_t[:], in_=idx1_in[:])
            fidx_t = cp.tile([P, FCH * 8], I16)
            nc.sync.dma_start(out=fidx_t[:], in_=fidx_in[:])
            rowloc_t = cp.tile([P, EC], F16)
            nc.sync.dma_start(out=rowloc_t[:], in_=rowloc_in[:])
            degcol_t = cp.tile([P, EC], F32)
            nc.sync.dma_start(out=degcol_t[:], in_=degcol_in[:])
            degsh_t = cp.tile([P, NT], F32)
            nc.sync.dma_start(out=degsh_t[:], in_=degsh_in[:])
            xs0_sb = cp.tile([P, NT, 192], F16)
            # zero-fill the pad columns of y1_bounce / h_l once
            zt = cp.tile([P, NT, D], F16)
            nc.gpsimd.memset(zt[:], 0.0)
            zdst1 = _dims(y1_bounce[:, 192:YW], [[YW, NSH], [1, YW - 192]])
            nc.sync.dma_start(out=zdst1, in_=zt[:])
            zdst2 = _dims(h_l[:, 576:HW], [[HW, NSH], [1, HW - 576]])
            nc.sync.dma_start(out=zdst2, in_=zt[:])

            def build_s(t):
                s_t = sp_.tile([P, NCHUNK, P], F16, tag="s")
                rl = rowloc_t[:, t * NCHUNK:(t + 1) * NCHUNK]
                rl_b = rl.to_broadcast([P, NCHUNK, P])
                io = iota_t[:]
                io_b = _dims(io, [io.ap[0], [0, NCHUNK], io.ap[1]])
                nc.vector.tensor_tensor(out=s_t[:], in0=rl_b, in1=io_b, op=eq)
                return s_t

            # whole-shard precomputes
            rq_all = cp.tile([P, 2, EC], F16)
            q32_all = cp.tile([P, EC], F32)
            nc.scalar.activation(q32_all[:], degcol_t[:], SQRT)
            nc.vector.tensor_copy(out=rq_all[:, 1, :], in_=q32_all[:])
            nc.vector.reciprocal(rq_all[:, 0, :], q32_all[:])
            degrev_all = cp.tile([P, NT], F32)
            nc.vector.reciprocal(degrev_all[:], degsh_t[:])
            rq0_all = cp.tile([P, 2, NT], F32)
            nc.scalar.activation(rq0_all[:, 1, :], degsh_t[:], SQRT)
            nc.vector.reciprocal(rq0_all[:, 0, :], rq0_all[:, 1, :])

            # ================= hop 1 =================
            with tc.tile_pool(name="v1", bufs=3) as v1p, \
                    tc.tile_pool(name="v3", bufs=3) as v3p:

                def h1_gather(g):
                    v = v1p.tile([P, 2 * GT * NL, D], F32, tag="v1")
                    b = g * GSLOTS
                    half = GT * TILE_SLOTS
                    nc.gpsimd.dma_gather(
                        v[:, 0:GT * NL, :], x_full[:],
                        idx1_t[:, b // 16:(b + half) // 16], half, half, D,
                        single_packet=False, queue_num=_q())
                    nc.gpsimd.dma_gather(
                        v[:, GT * NL:, :], x_full[LOCUT:, :],
                        idx1_t[:, (b + half) // 16:(b + 2 * half) // 16],
                        half, half, D, single_packet=False, queue_num=_q())
                    return v

                vq = {}
                if "hop1" not in ab:
                    vq[0] = h1_gather(0)
                    vq[1] = h1_gather(1)
                for g in range(NG if "hop1" not in ab else 0):
                    if g + 2 < NG:
                        vq[g + 2] = h1_gather(g + 2)
                    v = vq.pop(g)
                    for tau in range(GT):
                        t = GT * g + tau
                        s_t = build_s(t)
                        v3 = v3p.tile([P, NCHUNK, 192], F16, tag="v3")
                        # b0: gathered x values (lo block, hi block)
                        nc.scalar.activation(
                            v3[:, 0:NL, 0:D], v[:, NL * tau:NL * (tau + 1), :],
                            COPY)
                        nc.scalar.activation(
                            v3[:, NL:NCHUNK, 0:D],
                            v[:, GT * NL + NL * tau:GT * NL + NL * (tau + 1), :],
                            COPY)
                        # b12: x * {rsqrt(deg_col), sqrt(deg_col)} per half
                        for h in range(2):
                            dst = _dims(v3[:, h * NL:(h + 1) * NL, D:3 * D],
                                        [v3[:].ap[0], [192, NL], [D, 2], [1, D]])
                            src = _dims(v3[:, h * NL:(h + 1) * NL, 0:D],
                                        [v3[:].ap[0], [192, NL], [0, 2], [1, D]])
                            rq = rq_all[:, :, t * NCHUNK + h * NL:
                                        t * NCHUNK + (h + 1) * NL]
                            rqb = _dims(rq, [rq_all[:].ap[0], [1, NL],
                                             [EC, 2], [0, D]])
                            nc.vector.tensor_tensor(out=dst, in0=src, in1=rqb,
                                                    op=mult)
                        ps = pp.tile([P, 192], F32, space="PSUM", tag="ps")
                        for j in range(NCHUNK):
                            cc = NL * tau + j % NL + (GT * NL if j >= NL else 0)
                            nc.tensor.matmul(
                                out=ps[:], lhsT=s_t[:, j, :], rhs=v3[:, j, :],
                                start=(j == 0), stop=(j == NCHUNK - 1))
                        y1_t = evp.tile([P, 192], F16, tag="y1")
                        nc.scalar.activation(y1_t[:], ps[:], COPY,
                                             scale=degrev_all[:, t:t + 1])
                        ydst = _dims(y1_bounce[t * P:(t + 1) * P, 0:192],
                                     [[YW, P], [1, 192]])
                        nc.sync.dma_start(out=ydst, in_=y1_t[:])
                        hdst = _dims(h_l[t * P:(t + 1) * P, 192:384],
                                     [[HW, P], [1, 192]])
                        nc.sync.dma_start(out=hdst, in_=y1_t[:])
                        # xs0 = [x | x*rsqrt(deg_row) | x*sqrt(deg_row)]
                        x_t = x0p.tile([P, D], F32, tag="xt")
                        nc.sync.dma_start(out=x_t[:],
                                          in_=x_sh[t * P:(t + 1) * P, :])
                        nc.scalar.activation(xs0_sb[:, t, 0:D], x_t[:], COPY)
                        xb = _dims(x_t[:], [x_t[:].ap[0], [0, 2], [1, D]])
                        rq0b = _dims(rq0_all[:, :, t:t + 1],
                                     [rq0_all[:].ap[0], [NT, 2], [0, D]])
                        x12 = _dims(xs0_sb[:, t, D:3 * D],
                                    [xs0_sb[:].ap[0], [D, 2], [1, D]])
                        nc.vector.tensor_tensor(out=x12, in0=xb, in1=rq0b,
                                                op=mult)
                        h0dst = _dims(h_l[t * P:(t + 1) * P, 0:192],
                                      [[HW, P], [1, 192]])
                        nc.sync.dma_start(out=h0dst, in_=xs0_sb[:, t, :])
                    if "ag" not in ab:
                        nc.gpsimd.collective_compute(
                            "AllGather", mybir.AluOpType.bypass,
                            replica_groups=[list(range(NC))],
                            ins=[y1_bounce[g * GR:(g + 1) * GR, :]],
                            outs=[y1full[g * NC * GR:(g + 1) * NC * GR, :]],
                        )

            # ================= hop 2 =================
            # subgroups of (4, 3) tiles; v2 holds lo+hi chunks contiguously
            SUBS = [(0, 4), (4, 7)]
            with tc.tile_pool(name="v2", bufs=3) as v2p:

                def h2_gather(g, si):
                    t0, t1 = SUBS[si]
                    nch = NL * (t1 - t0)
                    v2 = v2p.tile([P, 2 * NL * 4, YW], F16, tag="v2")
                    b = g * GSLOTS
                    lo0 = b + t0 * TILE_SLOTS
                    hi0 = b + GT * TILE_SLOTS + t0 * TILE_SLOTS
                    n = nch * P
                    nc.gpsimd.dma_gather(
                        v2[:, 0:nch, :], y1full[:],
                        idx1_t[:, lo0 // 16:(lo0 + n) // 16], n, n, YW,
                        single_packet=False, queue_num=_q())
                    nc.gpsimd.dma_gather(
                        v2[:, nch:2 * nch, :], y1full[LOCUT:, :],
                        idx1_t[:, hi0 // 16:(hi0 + n) // 16], n, n, YW,
                        single_packet=False, queue_num=_q())
                    return v2

                pairs = [(g, si) for g in range(NG) for si in range(2)]
                v2q = {}
                if "hop2" not in ab:
                    v2q[pairs[0]] = h2_gather(*pairs[0])
                    v2q[pairs[1]] = h2_gather(*pairs[1])
                for pi, (g, si) in enumerate(pairs if "hop2" not in ab else []):
                    if pi + 2 < len(pairs):
                        v2q[pairs[pi + 2]] = h2_gather(*pairs[pi + 2])
                    v2 = v2q.pop((g, si))
                    t0, t1 = SUBS[si]
                    nch = NL * (t1 - t0)
                    for tau in range(t0, t1):
                        t = GT * g + tau
                        s_t = build_s(t)
                        ps = pp.tile([P, 192], F32, space="PSUM", tag="ps")
                        for j in range(NCHUNK):
                            cc = NL * (tau - t0) + j % NL + (nch if j >= NL else 0)
                            nc.tensor.matmul(
                                out=ps[:], lhsT=s_t[:, j, :],
                                rhs=v2[:, cc, 0:192],
                                start=(j == 0), stop=(j == NCHUNK - 1))
                        tmp = evp.tile([P, 192], F16, tag="tmp2")
                        nc.scalar.activation(tmp[:], ps[:], COPY,
                                             scale=degrev_all[:, t:t + 1])
                        xs2 = evp.tile([P, 192], F16, tag="xs2")
                        nc.vector.tensor_tensor(out=xs2[:], in0=tmp[:],
                                                in1=xs0_sb[:, t, :], op=sub)
                        h2dst = _dims(h_l[t * P:(t + 1) * P, 384:576],
                                      [[HW, P], [1, 192]])
                        nc.sync.dma_start(out=h2dst, in_=xs2[:])

            # ================= final gather =================
            with tc.tile_pool(name="g", bufs=2) as gp, \
                    tc.tile_pool(name="st", bufs=2) as stp:
                for p_i in range(FPC if "final" not in ab else 0):
                    gth = gp.tile([P, FCPP, HW], F16, tag="g")
                    n = FCPP * P
                    c0 = p_i * FCPP * 8
                    nc.gpsimd.dma_gather(
                        gth[:], h_l[:], fidx_t[:, c0:c0 + FCPP * 8], n, n, HW,
                        single_packet=False, queue_num=_q())
                    stage = stp.tile([P, FCPP, 576], F32, tag="st")
                    if p_i % 2 == 0:
                        nc.vector.tensor_copy(out=stage[:],
                                              in_=gth[:, :, 0:576])
                    else:
                        nc.scalar.activation(stage[:], gth[:, :, 0:576], COPY)
                    obase = out_f[p_i * FCPP * P:(p_i + 1) * FCPP * P, :]
                    orows = _dims(obase, [[576, P], [P * 576, FCPP], [1, 576]])
                    nc.sync.dma_start(out=orows, in_=stage[:])

    _split_multi_waits(nc)
    mybir.codegen_inst_isa_subclasses(nc)
    return nc


def _plan(x, deg, adj_row, adj_col, edge):
    """Host-side sharding: pure index bookkeeping + input reordering."""
    x = np.asarray(x, np.float32)
    deg = np.asarray(deg, np.float32).reshape(-1)
    adj_row = np.asarray(adj_row, np.int64)
    adj_col = np.asarray(adj_col, np.int64)
    edge = np.asarray(edge, np.int64)
    nnz = len(adj_row)

    # natural n -> gid = g*7168 + c*896 + r
    n_ids = np.arange(NPAD)
    c_of = n_ids // NSH
    loc_of = n_ids % NSH
    g_of = loc_of // GR
    r_of = loc_of % GR
    gid_of = g_of * (NC * GR) + c_of * GR + r_of

    x_full = np.zeros((NPAD, D), np.float32)
    x_full[gid_of[:N]] = x
    degp = np.ones(NPAD, np.float32)
    degp[:N] = deg
    iota_np = np.tile(np.arange(P, dtype=np.float16), (P, 1))

    # edge -> slot assignment (vectorized, global)
    t_glob = adj_row // P               # 0..391
    lane = (adj_row % P).astype(np.float16)
    gidc = gid_of[adj_col]
    is_hi = gidc >= LOCUT
    grp = t_glob * 2 + is_hi            # (tile, side) group id
    order = np.argsort(grp, kind="stable")
    counts = np.bincount(grp, minlength=NT * NC * 2)
    starts = np.concatenate(([0], np.cumsum(counts)))[:-1]
    rank = np.empty(nnz, np.int64)
    rank[order] = np.arange(nnz) - starts[grp[order]]
    assert counts.max() <= TILE_SLOTS, f"tile side overflow: {counts.max()}"

    core_e = t_glob // NT
    tcore = t_glob % NT
    g_e = tcore // GT
    tau_e = tcore % GT
    # stream position within the core
    pos = (g_e * GSLOTS + is_hi * (GT * TILE_SLOTS) + tau_e * TILE_SLOTS
           + rank)
    idx_val = (gidc - is_hi * LOCUT).astype(np.int16)
    # selector columns (tile-major)
    sel_col = tcore * NCHUNK + is_hi * NL + rank // P
    sel_p = rank % P

    def wrap(stream):
        w = stream.reshape(-1, 16).T.copy()
        return np.tile(w, (NC, 1))

    ep = edge.reshape(-1)
    ep_core = ep // NSH
    ep_loc = ep % NSH

    in_maps, positions = [], []
    for c in range(NC):
        m = core_e == c
        s1 = np.zeros(STREAM, np.int16)
        s1[pos[m]] = idx_val[m]
        rowloc = np.full((P, NT * NCHUNK), -1.0, np.float16)
        rowloc[sel_p[m], sel_col[m]] = lane[m]
        degcol = np.ones((P, NT * NCHUNK), np.float32)
        degcol[sel_p[m], sel_col[m]] = degp[adj_col[m]]

        dlocal = degp[c * NSH:(c + 1) * NSH]
        degsh = dlocal.reshape(NT, P).T.copy()
        x_shard = np.zeros((NSH, D), np.float32)
        real = min(NSH, max(0, N - c * NSH))
        x_shard[:real] = x[c * NSH:c * NSH + real]

        mine = np.nonzero(ep_core == c)[0]
        n_c = len(mine)
        assert n_c <= FCH * P, f"endpoint overflow: {n_c}"
        fs = np.zeros(FCH * P, np.int16)
        fs[:n_c] = ep_loc[mine].astype(np.int16)
        positions.append(mine)

        in_maps.append({
            "x_full": x_full,
            "x_sh": x_shard,
            "degsh": degsh,
            "idx1": wrap(s1),
            "fidx": wrap(fs),
            "rowloc": rowloc,
            "degcol": degcol,
            "iota": iota_np,
        })
    return in_maps, positions


def _assemble(results, positions):
    out = np.zeros((2 * EQ, 576), np.float32)
    for c in range(NC):
        rows = results[c]["out_f"]
        n_c = len(positions[c])
        out[positions[c]] = rows[:n_c]
    return out.reshape(2, EQ, 9, D).transpose(0, 1, 3, 2)


def kernel(x, deg, adj_row, adj_col, edge):
    import time
    if "nc" not in _prog_cache:
        t0 = time.time()
        _prog_cache["nc"] = _build_program()
        print(f"[kernel] program build: {time.time()-t0:.1f}s", flush=True)
    nc = _prog_cache["nc"]
    t0 = time.time()
    in_maps, positions = _plan(x, deg, adj_row, adj_col, edge)
    print(f"[kernel] host plan: {time.time()-t0:.1f}s", flush=True)
    t0 = time.time()
    res = run_bass_kernel_spmd(nc, in_maps, list(range(NC)))
    print(f"[kernel] compile+run: {time.time()-t0:.1f}s", flush=True)
    return _assemble(res.results, positions)


# revision 12
# speedup vs baseline: 3.4697x; 1.2407x over previous
"""Distributed Trainium2 Bass kernel for nn_NodeFeat (2-hop Chebyshev-style GNN
feature expansion + edge gather), 8 NeuronCores.

v3: batched SWDGE gathers on 4 parallel queues; uneven AllGather groups to
hide the collective tail; final gather bucketed by row-tile and overlapped
into hop2.

Structure:
  - 50000 nodes padded to 50176 = 8 x 6272; core c owns natural rows
    [6272c, 6272c+6272). Gather tables (x_full, y1full) use a permuted
    "gid" numbering gid = 8*G_ROFF[g] + c*G_ROWS[g] + r (g = AllGather
    group with G_TILES[g] tiles) so group-wise AllGathers land
    contiguously and one index space serves both hops.
  - Edges packed per 128-row tile into 18 chunks of 128 slots: 9 "lo"
    chunks (col gid < 25088) + 9 "hi" (>= 25088); the lo/hi split keeps
    dma_gather's int16 indices in range via a table base offset. Pad
    slots use idx 0 with selector rowloc=-1.
  - hop1: dma_gather fetches x[col] f32 rows per 3-tile subgroup (lo+hi
    pair, round-robin over 4 SWDGE queues); per-edge scaling
    {1, rsqrt(deg_col), sqrt(deg_col)} builds a fp16 moving operand;
    segment-sum on TensorE via one-hot selector matmuls in PSUM; ScalarE
    evacuates with the 1/deg row scale to fp16.
  - y1 stored fp16 in 256-wide rows (512B elems); group-wise AllGathers
    overlap hop1 compute; last group is 1 tile so its AG tail is tiny.
  - hop2: dma_gather fetches y1full fp16 rows (direct moving operand);
    same selectors; minus xs0 (kept resident in SBUF).
  - final: fused table h_l [6272, 640] fp16 = [xs0|y1|xs2|pad]; endpoints
    bucketed by row-tile range; each bucket's gather reads h_l[0:bound]
    only, so it runs as soon as its tiles' xs2 are written (inside hop2).
    Host reorders/transposes (bookkeeping).

All floating-point math runs on device; the host only shards, pads,
reorders and reassembles (index bookkeeping).
"""
import numpy as np

import concourse.bass as bass
import concourse.mybir as mybir
import concourse.tile as tile
from concourse import library_config
from concourse.bass_utils import run_bass_kernel_spmd

# ---------------- hardcoded problem geometry ----------------
N = 50000
D = 64
EQ = 32768
P = 128
NC = 8                   # cores
NT = 49                  # row tiles per core
NSH = NT * P             # 6272 rows per core
NPAD = NSH * NC          # 50176
G_TILES = [8, 8, 8, 8, 8, 8, 1]          # tiles per AllGather group
NG = len(G_TILES)
G_T0 = np.concatenate(([0], np.cumsum(G_TILES)))[:-1].tolist()
G_ROWS = [gt * P for gt in G_TILES]
G_ROFF = np.concatenate(([0], np.cumsum(G_ROWS)))[:-1].tolist()
NL = 9                   # lo chunks per tile
NH = 9                   # hi chunks per tile
NCHUNK = NL + NH         # 18
LOCUT = NPAD // 2        # 25088
TILE_SLOTS = NL * P      # 1152 slots per tile per side
SOFF = np.concatenate(
    ([0], np.cumsum([2 * gt * TILE_SLOTS for gt in G_TILES])))[:-1].tolist()
STREAM = 2 * NT * TILE_SLOTS             # 112896 slots per core
FB = [0, 8, 16, 24, 32, 40, 49]          # final bucket tile boundaries
NFB = len(FB) - 1                        # 6 buckets
FB_CAP = 13 * P                          # 1664 endpoint slots per bucket
FSUB = [7, 6]                            # chunks per sub-gather of a bucket
OUTR = NFB * FB_CAP                      # 9984 output rows per core
HW = 640                 # h_l row width (576 used + 64 pad, 1280B)
YW = 256                 # y1 row width fp16 (192 used + 64 pad, 512B)
F32 = mybir.dt.float32
F16 = mybir.dt.float16
I16 = mybir.dt.int16


def _subs(gt):
    """3-tile subgroups of a gt-tile group."""
    if gt == 1:
        return [(0, 1)]
    assert gt == 8
    return [(0, 3), (3, 6), (6, 8)]


_prog_cache = {}


class _TC(tile.TileContext):
    """TileContext whose final drain splits sem waits one-per-instruction
    (this walrus rejects >1 sync wait on an instruction)."""

    def _drain_and_barrier(self, tick_clock, wait_clock):
        nc = self.nc
        probe = nc.sync.nop()
        wait_clock.add_sem_waits(
            probe.ins, tile.ScopedClock({None: tick_clock.global_clock}))
        si = probe.ins.sync_info
        waits = list(si.on_wait) if si and si.on_wait else []
        if si is not None:
            si.on_wait = waits[:1]
        for w in waits[1:]:
            n2 = nc.sync.nop()
            if n2.ins.sync_info is None:
                n2.ins.sync_info = mybir.SyncInfo(on_wait=[w], on_update=[])
            else:
                n2.ins.sync_info.on_wait = [w]
        nc.sync.drain()
        nc.all_engine_barrier()
        popped = nc._tile_sem_poison_stack.pop()
        assert popped is self._sem_poison
        nc.clear_and_free_semaphores(list(self.sems.allocated().values()))
        nc.all_engine_barrier()


def _split_multi_waits(nc):
    for fn in nc.m.functions:
        for blk in fn.blocks:
            new_list = []
            for inst in blk.instructions:
                si = inst.sync_info
                waits = list(si.on_wait) if si and si.on_wait else []
                if len(waits) > 1:
                    for j, w in enumerate(waits[:-1]):
                        nop = mybir.InstNoOp(
                            name=f"{inst.name}-ws{j}",
                            engine=inst.engine,
                            ins=[], outs=[],
                            sync_info=mybir.SyncInfo(on_wait=[w], on_update=[]),
                        )
                        nc.register_instruction(nop, overwrite=True)
                        new_list.append(nop)
                    si.on_wait = waits[-1:]
                new_list.append(inst)
            blk.instructions[:] = new_list


def _dims(ap, dims):
    """Same tensor+offset as `ap`, explicit [stride(elem), nelem] dims."""
    return bass.AP(ap.tensor, ap.offset, dims)


def _build_program(ablate=()):
    """ablate: subset of {"hop1","ag","hop2","final"} to SKIP (perf
    ablation only -- results become wrong)."""
    ab = set(ablate)
    nc = bass.Bass("TRN2", target_bir_lowering=False, debug=False,
                   num_devices=NC, num_swdge_queues=4)

    x_full = nc.dram_tensor("x_full", [NPAD, D], F32, kind="ExternalInput")
    x_sh = nc.dram_tensor("x_sh", [NSH, D], F32, kind="ExternalInput")
    degsh_in = nc.dram_tensor("degsh", [P, NT], F32, kind="ExternalInput")
    idx1_in = nc.dram_tensor("idx1", [P, STREAM // 16], I16, kind="ExternalInput")
    fidx_in = nc.dram_tensor("fidx", [P, OUTR // 16], I16, kind="ExternalInput")
    rowloc_in = nc.dram_tensor("rowloc", [P, NT * NCHUNK], F16, kind="ExternalInput")
    degcol_in = nc.dram_tensor("degcol", [P, NT * NCHUNK], F32, kind="ExternalInput")
    iota_in = nc.dram_tensor("iota", [P, P], F16, kind="ExternalInput")

    out_f = nc.dram_tensor("out_f", [OUTR, 576], F32, kind="ExternalOutput")

    y1_bounce = nc.dram_tensor("y1_bounce", [NSH, YW], F16)
    y1full = nc.dram_tensor("y1full", [NPAD, YW], F16, addr_space="Shared")
    h_l = nc.dram_tensor("h_l", [NSH, HW], F16)

    eq = mybir.AluOpType.is_equal
    mult = mybir.AluOpType.mult
    sub = mybir.AluOpType.subtract
    COPY = mybir.ActivationFunctionType.Copy
    SQRT = mybir.ActivationFunctionType.Sqrt
    EC = NT * NCHUNK  # 882 selector columns

    qrr = [0]

    def _q():
        qrr[0] = (qrr[0] + 1) % 4
        return qrr[0]

    # subgroup pairs (g, t0, t1) in processing order, shared by both hops
    pairs = [(g, t0, t1) for g in range(NG) for (t0, t1) in _subs(G_TILES[g])]

    with _TC(nc) as tc, nc.allow_low_precision(
            reason="fp16 tables + matmul operands; PSUM accumulates in f32"), \
            nc.gpsimd.register("nreg_a") as nreg_a, \
            nc.gpsimd.register("nreg_b") as nreg_b, \
            nc.gpsimd.register("nreg_c") as nreg_c, \
            nc.gpsimd.register("nreg_d") as nreg_d, \
            nc.gpsimd.register("nreg_e") as nreg_e:
        nc.gpsimd.load_library(library_config.mlp)
        NREG = {3 * TILE_SLOTS: nreg_a, TILE_SLOTS: nreg_b,
                FSUB[0] * P: nreg_c, FSUB[1] * P: nreg_d,
                2 * TILE_SLOTS: nreg_e}
        for _nv, _nr in NREG.items():
            nc.gpsimd.reg_mov(_nr, _nv)
        with (
            tc.tile_pool(name="const", bufs=1) as cp,
            tc.tile_pool(name="s", bufs=3) as sp_,
            tc.tile_pool(name="ev", bufs=3) as evp,
            tc.tile_pool(name="x0", bufs=3) as x0p,
            tc.tile_pool(name="psum", bufs=4, space="PSUM") as pp,
        ):
            iota_t = cp.tile([P, P], F16)
            nc.sync.dma_start(out=iota_t[:], in_=iota_in[:])
            idx1_t = cp.tile([P, STREAM // 16], I16)
            nc.sync.dma_start(out=idx1_t[:], in_=idx1_in[:])
            fidx_t = cp.tile([P, OUTR // 16], I16)
            nc.sync.dma_start(out=fidx_t[:], in_=fidx_in[:])
            rowloc_t = cp.tile([P, EC], F16)
            nc.sync.dma_start(out=rowloc_t[:], in_=rowloc_in[:])
            degcol_t = cp.tile([P, EC], F32)
            nc.sync.dma_start(out=degcol_t[:], in_=degcol_in[:])
            degsh_t = cp.tile([P, NT], F32)
            nc.sync.dma_start(out=degsh_t[:], in_=degsh_in[:])
            xs0_sb = cp.tile([P, NT, 192], F16)
            # zero-fill the pad columns of y1_bounce / h_l once
            zt = cp.tile([P, NT, D], F16)
            nc.gpsimd.memset(zt[:], 0.0)
            zdst1 = _dims(y1_bounce[:, 192:YW], [[YW, NSH], [1, YW - 192]])
            nc.sync.dma_start(out=zdst1, in_=zt[:])
            zdst2 = _dims(h_l[:, 576:HW], [[HW, NSH], [1, HW - 576]])
            nc.sync.dma_start(out=zdst2, in_=zt[:])

            def build_s(t):
                s_t = sp_.tile([P, NCHUNK, P], F16, tag="s")
                rl = rowloc_t[:, t * NCHUNK:(t + 1) * NCHUNK]
                rl_b = rl.to_broadcast([P, NCHUNK, P])
                io = iota_t[:]
                io_b = _dims(io, [io.ap[0], [0, NCHUNK], io.ap[1]])
                nc.vector.tensor_tensor(out=s_t[:], in0=rl_b, in1=io_b, op=eq)
                return s_t

            # whole-shard precomputes
            rq_all = cp.tile([P, 2, EC], F16)
            q32_all = cp.tile([P, EC], F32)
            nc.scalar.activation(q32_all[:], degcol_t[:], SQRT)
            nc.vector.tensor_copy(out=rq_all[:, 1, :], in_=q32_all[:])
            nc.vector.reciprocal(rq_all[:, 0, :], q32_all[:])
            degrev_all = cp.tile([P, NT], F32)
            nc.vector.reciprocal(degrev_all[:], degsh_t[:])
            rq0_all = cp.tile([P, 2, NT], F32)
            nc.scalar.activation(rq0_all[:, 1, :], degsh_t[:], SQRT)
            nc.vector.reciprocal(rq0_all[:, 0, :], rq0_all[:, 1, :])

            # ================= hop 1 =================
            with tc.tile_pool(name="v1", bufs=3) as v1p, \
                    tc.tile_pool(name="v3", bufs=3) as v3p:

                def h1_gather(pi):
                    g, t0, t1 = pairs[pi]
                    gt = G_TILES[g]
                    ntile = t1 - t0
                    nch = NL * ntile
                    n = ntile * TILE_SLOTS
                    v = v1p.tile([P, 2 * NL * 3, D], F32, tag="v1")
                    lo0 = SOFF[g] + t0 * TILE_SLOTS
                    hi0 = SOFF[g] + gt * TILE_SLOTS + t0 * TILE_SLOTS
                    nc.gpsimd.dma_gather(
                        v[:, 0:nch, :], x_full[:],
                        idx1_t[:, lo0 // 16:(lo0 + n) // 16], n, NREG[n], D,
                        single_packet=False, queue_num=_q())
                    nc.gpsimd.dma_gather(
                        v[:, nch:2 * nch, :], x_full[LOCUT:, :],
                        idx1_t[:, hi0 // 16:(hi0 + n) // 16], n, NREG[n], D,
                        single_packet=False, queue_num=_q())
                    return v

                vq = {}
                if "hop1" not in ab:
                    vq[0] = h1_gather(0)
                    vq[1] = h1_gather(1)
                for pi, (g, t0, t1) in enumerate(
                        pairs if "hop1" not in ab else []):
                    if pi + 2 < len(pairs):
                        vq[pi + 2] = h1_gather(pi + 2)
                    v = vq.pop(pi)
                    nch = NL * (t1 - t0)
                    for tau in range(t0, t1):
                        t = G_T0[g] + tau
                        s_t = build_s(t)
                        v3 = v3p.tile([P, NCHUNK, 192], F16, tag="v3")
                        co = NL * (tau - t0)
                        # b0: gathered x values (lo block, hi block)
                        nc.scalar.activation(
                            v3[:, 0:NL, 0:D], v[:, co:co + NL, :], COPY)
                        nc.scalar.activation(
                            v3[:, NL:NCHUNK, 0:D],
                            v[:, nch + co:nch + co + NL, :], COPY)
                        # b12: x * {rsqrt(deg_col), sqrt(deg_col)} per half
                        for h in range(2):
                            dst = _dims(v3[:, h * NL:(h + 1) * NL, D:3 * D],
                                        [v3[:].ap[0], [192, NL], [D, 2], [1, D]])
                            src = _dims(v3[:, h * NL:(h + 1) * NL, 0:D],
                                        [v3[:].ap[0], [192, NL], [0, 2], [1, D]])
                            rq = rq_all[:, :, t * NCHUNK + h * NL:
                                        t * NCHUNK + (h + 1) * NL]
                            rqb = _dims(rq, [rq_all[:].ap[0], [1, NL],
                                             [EC, 2], [0, D]])
                            nc.vector.tensor_tensor(out=dst, in0=src, in1=rqb,
                                                    op=mult)
                        ps = pp.tile([P, 192], F32, space="PSUM", tag="ps")
                        for j in range(NCHUNK):
                            nc.tensor.matmul(
                                out=ps[:], lhsT=s_t[:, j, :], rhs=v3[:, j, :],
                                start=(j == 0), stop=(j == NCHUNK - 1))
                        y1_t = evp.tile([P, 192], F16, tag="y1")
                        nc.scalar.activation(y1_t[:], ps[:], COPY,
                                             scale=degrev_all[:, t:t + 1])
                        ydst = _dims(y1_bounce[t * P:(t + 1) * P, 0:192],
                                     [[YW, P], [1, 192]])
                        nc.sync.dma_start(out=ydst, in_=y1_t[:])
                        hdst = _dims(h_l[t * P:(t + 1) * P, 192:384],
                                     [[HW, P], [1, 192]])
                        nc.sync.dma_start(out=hdst, in_=y1_t[:])
                        # xs0 = [x | x*rsqrt(deg_row) | x*sqrt(deg_row)]
                        x_t = x0p.tile([P, D], F32, tag="xt")
                        nc.sync.dma_start(out=x_t[:],
                                          in_=x_sh[t * P:(t + 1) * P, :])
                        nc.scalar.activation(xs0_sb[:, t, 0:D], x_t[:], COPY)
                        xb = _dims(x_t[:], [x_t[:].ap[0], [0, 2], [1, D]])
                        rq0b = _dims(rq0_all[:, :, t:t + 1],
                                     [rq0_all[:].ap[0], [NT, 2], [0, D]])
                        x12 = _dims(xs0_sb[:, t, D:3 * D],
                                    [xs0_sb[:].ap[0], [D, 2], [1, D]])
                        nc.vector.tensor_tensor(out=x12, in0=xb, in1=rq0b,
                                                op=mult)
                        h0dst = _dims(h_l[t * P:(t + 1) * P, 0:192],
                                      [[HW, P], [1, 192]])
                        nc.sync.dma_start(out=h0dst, in_=xs0_sb[:, t, :])
                    if t1 == G_TILES[g] and "ag" not in ab:
                        nc.gpsimd.collective_compute(
                            "AllGather", mybir.AluOpType.bypass,
                            replica_groups=[list(range(NC))],
                            ins=[y1_bounce[G_ROFF[g]:G_ROFF[g] + G_ROWS[g], :]],
                            outs=[y1full[NC * G_ROFF[g]:
                                         NC * (G_ROFF[g] + G_ROWS[g]), :]],
                        )

            # ================= hop 2 (+ final buckets) =================
            with tc.tile_pool(name="v2", bufs=3) as v2p, \
                    tc.tile_pool(name="g", bufs=2) as gp, \
                    tc.tile_pool(name="st", bufs=2) as stp:

                def h2_gather(pi):
                    g, t0, t1 = pairs[pi]
                    gt = G_TILES[g]
                    ntile = t1 - t0
                    nch = NL * ntile
                    n = ntile * TILE_SLOTS
                    v2 = v2p.tile([P, 2 * NL * 3, YW], F16, tag="v2")
                    lo0 = SOFF[g] + t0 * TILE_SLOTS
                    hi0 = SOFF[g] + gt * TILE_SLOTS + t0 * TILE_SLOTS
                    nc.gpsimd.dma_gather(
                        v2[:, 0:nch, :], y1full[:],
                        idx1_t[:, lo0 // 16:(lo0 + n) // 16], n, NREG[n], YW,
                        single_packet=False, queue_num=_q())
                    nc.gpsimd.dma_gather(
                        v2[:, nch:2 * nch, :], y1full[LOCUT:, :],
                        idx1_t[:, hi0 // 16:(hi0 + n) // 16], n, NREG[n], YW,
                        single_packet=False, queue_num=_q())
                    return v2

                def emit_bucket(b):
                    bound = FB[b + 1] * P
                    ch0 = 0
                    for si, nchk in enumerate(FSUB):
                        gth = gp.tile([P, FSUB[0], HW], F16, tag="g")
                        scol = (b * (FB_CAP // P) + ch0) * 8
                        n = nchk * P
                        nc.gpsimd.dma_gather(
                            gth[:, 0:nchk, :], h_l[0:bound, :],
                            fidx_t[:, scol:scol + n // 16], n, NREG[n], HW,
                            single_packet=False, queue_num=_q())
                        stage = stp.tile([P, FSUB[0], 576], F32, tag="st")
                        if si % 2 == 0:
                            nc.vector.tensor_copy(out=stage[:, 0:nchk, :],
                                                  in_=gth[:, 0:nchk, 0:576])
                        else:
                            nc.scalar.activation(stage[:, 0:nchk, :],
                                                 gth[:, 0:nchk, 0:576], COPY)
                        rbase = b * FB_CAP + ch0 * P
                        obase = out_f[rbase:rbase + n, :]
                        orows = _dims(obase,
                                      [[576, P], [P * 576, nchk], [1, 576]])
                        nc.sync.dma_start(out=orows, in_=stage[:, 0:nchk, :])
                        ch0 += nchk

                v2q = {}
                if "hop2" not in ab:
                    v2q[0] = h2_gather(0)
                    v2q[1] = h2_gather(1)
                for pi, (g, t0, t1) in enumerate(
                        pairs if "hop2" not in ab else []):
                    if pi + 2 < len(pairs):
                        v2q[pi + 2] = h2_gather(pi + 2)
                    v2 = v2q.pop(pi)
                    nch = NL * (t1 - t0)
                    for tau in range(t0, t1):
                        t = G_T0[g] + tau
                        s_t = build_s(t)
                        ps = pp.tile([P, 192], F32, space="PSUM", tag="ps")
                        for j in range(NCHUNK):
                            cc = NL * (tau - t0) + j % NL + (nch if j >= NL else 0)
                            nc.tensor.matmul(
                                out=ps[:], lhsT=s_t[:, j, :],
                                rhs=v2[:, cc, 0:192],
                                start=(j == 0), stop=(j == NCHUNK - 1))
                        tmp = evp.tile([P, 192], F16, tag="tmp2")
                        nc.scalar.activation(tmp[:], ps[:], COPY,
                                             scale=degrev_all[:, t:t + 1])
                        xs2 = evp.tile([P, 192], F16, tag="xs2")
                        nc.vector.tensor_tensor(out=xs2[:], in0=tmp[:],
                                                in1=xs0_sb[:, t, :], op=sub)
                        h2dst = _dims(h_l[t * P:(t + 1) * P, 384:576],
                                      [[HW, P], [1, 192]])
                        nc.sync.dma_start(out=h2dst, in_=xs2[:])
                    if "final" not in ab:
                        t_done = G_T0[g] + t1
                        for b in range(NFB):
                            if FB[b + 1] == t_done:
                                emit_bucket(b)

    _split_multi_waits(nc)
    mybir.codegen_inst_isa_subclasses(nc)
    return nc


def _plan(x, deg, adj_row, adj_col, edge):
    """Host-side sharding: pure index bookkeeping + input reordering."""
    x = np.asarray(x, np.float32)
    deg = np.asarray(deg, np.float32).reshape(-1)
    adj_row = np.asarray(adj_row, np.int64)
    adj_col = np.asarray(adj_col, np.int64)
    edge = np.asarray(edge, np.int64)
    nnz = len(adj_row)

    g_t0 = np.asarray(G_T0)
    g_rows = np.asarray(G_ROWS)
    g_roff = np.asarray(G_ROFF)
    soff = np.asarray(SOFF)
    g_of_t = np.searchsorted(np.cumsum(G_TILES), np.arange(NT), side="right")

    # natural n -> gid
    n_ids = np.arange(NPAD)
    c_of = n_ids // NSH
    loc_of = n_ids % NSH
    t_of = loc_of // P
    lane_of = loc_of % P
    g_of = g_of_t[t_of]
    r_of = (t_of - g_t0[g_of]) * P + lane_of
    gid_of = NC * g_roff[g_of] + c_of * g_rows[g_of] + r_of

    x_full = np.zeros((NPAD, D), np.float32)
    x_full[gid_of[:N]] = x
    degp = np.ones(NPAD, np.float32)
    degp[:N] = deg
    iota_np = np.tile(np.arange(P, dtype=np.float16), (P, 1))

    # edge -> slot assignment (vectorized, global)
    t_glob = adj_row // P               # 0..391
    lane = (adj_row % P).astype(np.float16)
    gidc = gid_of[adj_col]
    is_hi = gidc >= LOCUT
    grp = t_glob * 2 + is_hi
    order = np.argsort(grp, kind="stable")
    counts = np.bincount(grp, minlength=NT * NC * 2)
    starts = np.concatenate(([0], np.cumsum(counts)))[:-1]
    rank = np.empty(nnz, np.int64)
    rank[order] = np.arange(nnz) - starts[grp[order]]
    assert counts.max() <= TILE_SLOTS, f"tile side overflow: {counts.max()}"

    core_e = t_glob // NT
    tcore = t_glob % NT
    g_e = g_of_t[tcore]
    pos = (soff[g_e] + is_hi * (np.asarray(G_TILES)[g_e] * TILE_SLOTS)
           + (tcore - g_t0[g_e]) * TILE_SLOTS + rank)
    idx_val = (gidc - is_hi * LOCUT).astype(np.int16)
    sel_col = tcore * NCHUNK + is_hi * NL + rank // P
    sel_p = rank % P

    def wrap(stream):
        w = stream.reshape(-1, 16).T.copy()
        return np.tile(w, (NC, 1))

    # final endpoint bucketing by row-tile range
    fb_of_t = np.searchsorted(np.asarray(FB[1:]), np.arange(NT), side="right")
    ep = edge.reshape(-1)
    ep_core = ep // NSH
    ep_loc = ep % NSH
    ep_b = fb_of_t[ep_loc // P]

    in_maps, positions = [], []
    for c in range(NC):
        m = core_e == c
        s1 = np.zeros(STREAM, np.int16)
        s1[pos[m]] = idx_val[m]
        rowloc = np.full((P, NT * NCHUNK), -1.0, np.float16)
        rowloc[sel_p[m], sel_col[m]] = lane[m]
        degcol = np.ones((P, NT * NCHUNK), np.float32)
        degcol[sel_p[m], sel_col[m]] = degp[adj_col[m]]

        dlocal = degp[c * NSH:(c + 1) * NSH]
        degsh = dlocal.reshape(NT, P).T.copy()
        x_shard = np.zeros((NSH, D), np.float32)
        real = min(NSH, max(0, N - c * NSH))
        x_shard[:real] = x[c * NSH:c * NSH + real]

        mc = ep_core == c
        bidx = ep_b[mc]
        eidx = np.nonzero(mc)[0]
        bord = np.argsort(bidx, kind="stable")
        bcnt = np.bincount(bidx, minlength=NFB)
        assert bcnt.max() <= FB_CAP, f"bucket overflow: {bcnt.max()}"
        bstart = np.concatenate(([0], np.cumsum(bcnt)))[:-1]
        brank = np.empty(len(bidx), np.int64)
        brank[bord] = np.arange(len(bidx)) - bstart[bidx[bord]]
        fpos = bidx * FB_CAP + brank
        fs = np.zeros(OUTR, np.int16)
        fs[fpos] = ep_loc[eidx].astype(np.int16)
        positions.append((eidx, fpos))

        in_maps.append({
            "x_full": x_full,
            "x_sh": x_shard,
            "degsh": degsh,
            "idx1": wrap(s1),
            "fidx": wrap(fs),
            "rowloc": rowloc,
            "degcol": degcol,
            "iota": iota_np,
        })
    return in_maps, positions


def _assemble(results, positions):
    out = np.zeros((2 * EQ, 576), np.float32)
    for c in range(NC):
        rows = results[c]["out_f"]
        eidx, fpos = positions[c]
        out[eidx] = rows[fpos]
    return out.reshape(2, EQ, 9, D).transpose(0, 1, 3, 2)


def kernel(x, deg, adj_row, adj_col, edge):
    import time
    if "nc" not in _prog_cache:
        t0 = time.time()
        _prog_cache["nc"] = _build_program()
        print(f"[kernel] program build: {time.time()-t0:.1f}s", flush=True)
    nc = _prog_cache["nc"]
    t0 = time.time()
    in_maps, positions = _plan(x, deg, adj_row, adj_col, edge)
    print(f"[kernel] host plan: {time.time()-t0:.1f}s", flush=True)
    t0 = time.time()
    res = run_bass_kernel_spmd(nc, in_maps, list(range(NC)))
    print(f"[kernel] compile+run: {time.time()-t0:.1f}s", flush=True)
    return _assemble(res.results, positions)
